# revision 18
# baseline (speedup 1.0000x reference)
"""Trainium2 Bass kernel for nn_BBoxGenerator (segment_reduce).

mask_fg (256, 1, 512, 512) f32 -> boxes (256, 4) f32 [x0, y0, x1, y1].

Pure data parallel: each of the 8 cores handles 32 images independently.

v4 design (from trace analysis of v1 at 109.3us measured):
  - The 16 SDMA engines are the stream floor (~27 GB/s each, ~84us for
    32 MiB); they ran gap-free in v1. v4 attacks the head (stream start
    10.1us -> ~7.6us via a tiny first DMA piece + affine_select one-hot
    instead of 33 Q7 memsets) and the tail (v1: 15.5us of serial
    finishing after the last byte).
  - Unified "positive iff any" masks across both threshold engines:
    DVE rows use (x > 0.5) in {0,1} with fused accum row sums; ACT rows
    use Relu(x - 0.5) in [0, 0.5) with fused accum row sums. Sums and
    PSUM column counts are then all "> 0.002 iff any foreground" - one
    threshold everywhere, one one-hot, PSUM rows 0..31 in image order,
    ONE output DMA (no un-permute). (v3's single-engine reduce-max was
    2.2us/image on DVE - tensor_reduce runs ~1 elem/cycle - which made
    Vector the stream limiter at 126us.)
  - Col side: counts in PSUM -> fused (cnt > 0.5) * iota via
    scalar_tensor_tensor + tensor_reduce min/max straight into the box
    tile. (tensor_tensor_reduce would fuse further but crashes the HW:
    NRT_EXEC_UNIT_UNRECOVERABLE, bisected in micro_test.py stage 5.)
  - Row side: per-group masked-iota min/max into rvals during the
    stream (groups 8,8,8,7,1 so the last group is one image); one PE
    transpose + two small reduces at the tail.
  - Box expand fused without predicates:
    lo' = min(lo, max(0, c - s/2)), hi' = max(hi, min(1, c + s/2)).
  - Last two images stream in quarter-image DMA pieces with
    per-quarter threshold/reduce so tail latency tracks the last
    arriving bytes.
"""

import numpy as np

from concourse import bacc, mybir
from concourse.tile import TileContext
from concourse.bass_utils import run_bass_kernel_spmd

F32 = mybir.dt.float32
BF16 = mybir.dt.bfloat16
I32 = mybir.dt.int32
OP = mybir.AluOpType
AX = mybir.AxisListType
AF = mybir.ActivationFunctionType

N_CORES = 8
B = 256
BP = B // N_CORES  # 32 images per core
H = W = 512
GROUP = 8  # images per row-side finishing group
NG = BP // GROUP

MIN_BOX = 0.05


def build_nc():
    nc = bacc.Bacc("TRN2", target_bir_lowering=False, debug=False, num_devices=N_CORES)
    x = nc.declare_dram_parameter("mask_fg", [BP, 1, H, W], F32, isOutput=False)
    out = nc.declare_dram_parameter("out", [BP, 4], F32, isOutput=True)

    # (128, BP, 4, 512): partition p holds rows 4p..4p+3 of each image
    xv = x.ap().rearrange("b one (p a) w -> p (b one) a w", p=128)

    with TileContext(nc) as tc:
        with (
            tc.tile_pool(name="consts", bufs=1) as consts,
            tc.tile_pool(name="imgs", bufs=22) as imgs,
            tc.tile_pool(name="masks", bufs=8) as masks,
            tc.tile_pool(name="small", bufs=1) as small,
            tc.tile_pool(name="pcol", bufs=1, space="PSUM") as pcol_pool,
            tc.tile_pool(name="ptr", bufs=1, space="PSUM") as ptr_pool,
        ):
            psum_col = pcol_pool.tile([BP, W], F32)
            oh = consts.tile([128, BP * BP], BF16)
            ones_oh = consts.tile([128, BP * BP], BF16)
            any_t = small.tile([128, 4 * BP], F32)
            rvals = small.tile([128, 2 * BP], F32)

            # late consts (declared up front, emitted mid-stream on Q7)
            neg_half = consts.tile([128, 1], F32)
            hm_lo_i = consts.tile([128, GROUP * 4], I32)
            hm_lo = consts.tile([128, GROUP * 4], F32)
            hm_hi_i = consts.tile([128, GROUP * 4], I32)
            hm_hi = consts.tile([128, GROUP * 4], F32)
            wm_lo_i = consts.tile([BP, W], I32)
            wm_lo = consts.tile([BP, W], F32)
            wm_hi_i = consts.tile([BP, W], I32)
            wm_hi = consts.tile([BP, W], F32)
            ones128 = consts.tile([128, 128], F32)
            ident = consts.tile([128, 128], F32)
            dflt = consts.tile([BP, 4], F32)

            def emit_oh():
                nc.gpsimd.memset(neg_half[:], -0.5)
                # OH[p, i*32 + j] = (i == j): routes image i to PSUM row i
                nc.gpsimd.memset(ones_oh[:], 1.0)
                nc.gpsimd.affine_select(
                    oh[:], ones_oh[:], [[-1, BP], [1, BP]], OP.is_equal, 0.0,
                    base=0, channel_multiplier=0,
                )

            def emit_late_consts():
                # row-index tables: y = 4p + r; lo = y - 512, hi = y + 1
                nc.gpsimd.iota(hm_lo_i[:], [[0, GROUP], [1, 4]], base=-512,
                               channel_multiplier=4)
                nc.gpsimd.tensor_copy(hm_lo[:], hm_lo_i[:])
                nc.gpsimd.iota(hm_hi_i[:], [[0, GROUP], [1, 4]], base=1,
                               channel_multiplier=4)
                nc.gpsimd.tensor_copy(hm_hi[:], hm_hi_i[:])
                # col-index tables: lo = j - 512, hi = j + 1
                nc.gpsimd.iota(wm_lo_i[:], [[1, W]], base=-512, channel_multiplier=0)
                nc.gpsimd.tensor_copy(wm_lo[:], wm_lo_i[:])
                nc.gpsimd.iota(wm_hi_i[:], [[1, W]], base=1, channel_multiplier=0)
                nc.gpsimd.tensor_copy(wm_hi[:], wm_hi_i[:])
                nc.gpsimd.memset(ones128[:], 1.0)
                nc.gpsimd.affine_select(
                    ident[:], ones128[:], [[-1, 128]], OP.is_equal, 0.0,
                    base=0, channel_multiplier=1,
                )
                nc.gpsimd.memset(dflt[:, 0:2], 0.25)
                nc.gpsimd.memset(dflt[:, 2:4], 0.75)

            # DMA piece schedule: img -> list of (a_lo, a_hi) row-block pieces.
            # First image starts tiny so the SDMA stream begins ASAP; the last
            # two stream in quarters to cut tail latency.
            def dma_pieces(i):
                if i == 0:
                    return [(0, 1), (1, 4)]
                if i >= BP - 2:
                    return [(0, 1), (1, 2), (2, 3), (3, 4)]
                return [(0, 4)]

            # per-image row-block engine split: 'd' = DVE is_gt {0,1} count,
            # 'a' = ACT Relu(x-0.5) >= 0 sum. Both make any_t/PSUM entries
            # "> ANY_THR iff any foreground". For the last two images the
            # late-arriving row blocks go to DVE (shorter latency, and DVE
            # runs the rest of the tail anyway).
            def row_engines(i):
                if i >= BP - 2:
                    return "aadd"
                return "ddaa"

            # row-side groups: last group is a single image so the final
            # rvals contribution is tiny
            row_groups = [(0, 8), (8, 8), (16, 8), (24, 7), (31, 1)]

            ANY_THR = 0.002  # any_t/PSUM entries exceed this iff any foreground

            def finish_group(start, n):
                # masked row-index min/max for images start..start+n-1
                cs = slice(4 * start, 4 * (start + n))
                rt_lo = small.tile([128, 4 * GROUP], F32, tag="rt_lo")
                nc.vector.scalar_tensor_tensor(
                    rt_lo[:, 0:4 * n], any_t[:, cs], ANY_THR, hm_lo[:, 0:4 * n],
                    OP.is_gt, OP.mult)
                nc.vector.tensor_reduce(
                    rvals[:, start:start + n],
                    rt_lo[:, 0:4 * n].rearrange("p (i r) -> p i r", r=4),
                    op=OP.min, axis=AX.X)
                rt_hi = small.tile([128, 4 * GROUP], F32, tag="rt_hi")
                nc.vector.scalar_tensor_tensor(
                    rt_hi[:, 0:4 * n], any_t[:, cs], ANY_THR, hm_hi[:, 0:4 * n],
                    OP.is_gt, OP.mult)
                nc.vector.tensor_reduce(
                    rvals[:, BP + start:BP + start + n],
                    rt_hi[:, 0:4 * n].rearrange("p (i r) -> p i r", r=4),
                    op=OP.max, axis=AX.X)

            # ---- main loop: one image at a time ----
            def emit_dma(i):
                img = imgs.tile([128, 4, W], BF16, tag="img")
                for (al, ah) in dma_pieces(i):
                    nc.gpsimd.dma_start(out=img[:, al:ah, :], in_=xv[:, i:i + 1, al:ah])
                return img

            def emit_compute(i, img):
                m01 = masks.tile([128, 4, W], BF16, tag="m01")
                for r, eng in enumerate(row_engines(i)):
                    acc = any_t[:, 4 * i + r:4 * i + r + 1]
                    if eng == "d":
                        nc.vector.tensor_scalar(
                            m01[:, r, :], img[:, r, :], 0.5, None,
                            OP.is_gt, OP.add, accum_out=acc)
                    else:
                        nc.scalar.activation(
                            m01[:, r, :], img[:, r, :], AF.Relu,
                            bias=neg_half[:], accum_out=acc)
                for r in range(4):
                    nc.tensor.matmul(
                        psum_col[:, :], oh[:, i * BP:(i + 1) * BP], m01[:, r, :],
                        start=(i == 0 and r == 0), stop=(i == BP - 1 and r == 3),
                    )
                for (gs, gn) in row_groups:
                    if gs + gn - 1 == i:
                        finish_group(gs, gn)

            # DMAs for images 0-1 first, then ALL Q7 const work in one early
            # block: the SWDGE ring is still shallow here (emission ahead of
            # drain), while any Q7 compute op later in the loop stalls
            # descriptor emission ring-full and gaps the whole stream. The
            # consts still precede every consumer in program order.
            img01 = [emit_dma(0), emit_dma(1)]
            emit_oh()
            emit_late_consts()
            emit_compute(0, img01[0])
            emit_compute(1, img01[1])
            for i in range(2, BP):
                img = emit_dma(i)
                emit_compute(i, img)

            # ---- tail ----
            braw = small.tile([BP, 4], F32)

            # col side: (colsum > thr) * iota, masked min/max
            scr_lo = small.tile([BP, W], F32)
            scr_hi = small.tile([BP, W], F32)
            nc.vector.scalar_tensor_tensor(
                scr_lo[:], psum_col[:, :], ANY_THR, wm_lo[:], OP.is_gt, OP.mult)
            nc.vector.tensor_reduce(braw[:, 0:1], scr_lo[:], op=OP.min, axis=AX.X)
            nc.vector.scalar_tensor_tensor(
                scr_hi[:], psum_col[:, :], ANY_THR, wm_hi[:], OP.is_gt, OP.mult)
            nc.vector.tensor_reduce(braw[:, 2:3], scr_hi[:], op=OP.max, axis=AX.X)

            # row side: transpose rvals, reduce per image
            rT = ptr_pool.tile([2 * BP, 128], F32)
            nc.tensor.transpose(rT[:], rvals[:], ident[:])
            nc.vector.tensor_reduce(braw[:, 1:2], rT[0:BP, :], op=OP.min, axis=AX.X)
            nc.vector.tensor_reduce(braw[:, 3:4], rT[BP:2 * BP, :], op=OP.max, axis=AX.X)

            # empty mask: y_max raw is 0 iff no foreground
            emp = small.tile([BP, 1], F32)
            nc.vector.tensor_scalar(emp[:], braw[:, 3:4], 0.5, None, OP.is_lt)

            # normalize: lo = (v + 512)/512, hi = (v - 1)/512
            boxes = small.tile([BP, 4], F32)
            nc.vector.tensor_scalar(
                boxes[:, 0:2], braw[:, 0:2], 512.0, 1.0 / 512, OP.add, OP.mult)
            nc.vector.tensor_scalar(
                boxes[:, 2:4], braw[:, 2:4], 1.0, 1.0 / 512, OP.subtract, OP.mult)

            # expand-small fused: lo' = min(lo, max(0, c - s/2)),
            #                     hi' = max(hi, min(1, c + s/2))
            csum = small.tile([BP, 2], F32)
            lo2 = small.tile([BP, 2], F32)
            hi2 = small.tile([BP, 2], F32)
            final = small.tile([BP, 4], F32)
            nc.vector.tensor_add(csum[:], boxes[:, 0:2], boxes[:, 2:4])
            nc.vector.tensor_scalar(
                lo2[:], csum[:], 0.5, MIN_BOX * 0.5, OP.mult, OP.subtract)
            nc.vector.tensor_scalar(lo2[:], lo2[:], 0.0, None, OP.max)
            nc.vector.tensor_tensor(final[:, 0:2], boxes[:, 0:2], lo2[:], op=OP.min)
            nc.vector.tensor_scalar(
                hi2[:], csum[:], 0.5, MIN_BOX * 0.5, OP.mult, OP.add)
            nc.vector.tensor_scalar(hi2[:], hi2[:], 1.0, None, OP.min)
            nc.vector.tensor_tensor(final[:, 2:4], boxes[:, 2:4], hi2[:], op=OP.max)

            # default box where empty: final += (default - final) * emp
            dmb = small.tile([BP, 4], F32)
            nc.vector.tensor_sub(dmb[:], dflt[:], final[:])
            outb = small.tile([BP, 4], F32)
            nc.vector.scalar_tensor_tensor(
                outb[:], dmb[:], emp[:], final[:], OP.mult, OP.add)

            nc.sync.dma_start(out=out.ap(), in_=outb[:])

    return nc


_NC = None


def _get_nc():
    global _NC
    if _NC is None:
        nc = build_nc()
        nc.compile()
        _NC = nc
    return _NC


def kernel(mask_fg: np.ndarray) -> np.ndarray:
    mask_fg = np.ascontiguousarray(np.asarray(mask_fg, dtype=np.float32))
    assert mask_fg.shape == (B, 1, H, W), mask_fg.shape
    nc = _get_nc()
    shards = mask_fg.reshape(N_CORES, BP, 1, H, W)
    in_maps = [{"mask_fg": np.ascontiguousarray(shards[i])} for i in range(N_CORES)]
    res = run_bass_kernel_spmd(nc, in_maps, core_ids=list(range(N_CORES)))
    return np.concatenate(
        [res.results[i]["out"] for i in range(N_CORES)], axis=0
    ).astype(np.float32)


# revision 19
# speedup vs baseline: 1.0119x; 1.0119x over previous
"""Trainium2 Bass kernel for nn_BBoxGenerator (segment_reduce).

mask_fg (256, 1, 512, 512) f32 -> boxes (256, 4) f32 [x0, y0, x1, y1].

Pure data parallel: each of the 8 cores handles 32 images independently.

v7: v1's stream machinery (2D tiles, one 1 MiB SWDGE casting DMA per
image, 64-wide one-hot stationary) + reworked compute/finishing:
  - Unified "positive iff any" masks: DVE rows use (x > 0.5) {0,1} with
    fused accum row sums; ACT rows use Relu(x-0.5) >= 0 with fused accum
    row sums. Row sums and PSUM column sums are all "> 0.002 iff any
    foreground": one threshold everywhere, PSUM rows 0..31 in image
    order, ONE output DMA (no un-permute).
  - Row side: per-group masked-iota min/max into rvals during the
    stream; one PE transpose + two small reduces at the tail.
  - Col side: fused (colsum > thr) * iota via scalar_tensor_tensor +
    tensor_reduce min/max. (tensor_tensor_reduce would fuse further but
    crashes the HW: NRT_EXEC_UNIT_UNRECOVERABLE, micro_test.py stage 5.)
  - Box expand fused without predicates:
    lo' = min(lo, max(0, c - s/2)), hi' = max(hi, min(1, c + s/2)).
  - Q7 (gpsimd) paces SWDGE descriptor emission ring-full, so const
    emission is split: tiny block (one-hot + row tables) right after
    dma1 while the ring is shallow; tail-only consts before dma30 where
    the deep ring absorbs the pause.
"""

import numpy as np

from concourse import bacc, mybir
from concourse.tile import TileContext
from concourse.bass_utils import run_bass_kernel_spmd

F32 = mybir.dt.float32
BF16 = mybir.dt.bfloat16
I32 = mybir.dt.int32
OP = mybir.AluOpType
AX = mybir.AxisListType
AF = mybir.ActivationFunctionType

N_CORES = 8
B = 256
BP = B // N_CORES  # 32 images per core
H = W = 512
IMG_FREE = 4 * W  # 2048 free elems per image (4 rows per partition)
GROUP = 8
OHW = 64  # one-hot block width per image (v1 width; rows 32..63 unused)

MIN_BOX = 0.05
ANY_THR = 0.002  # any_t/PSUM sums exceed this iff any foreground


def build_nc():
    nc = bacc.Bacc("TRN2", target_bir_lowering=False, debug=False, num_devices=N_CORES)
    x = nc.declare_dram_parameter("mask_fg", [BP, 1, H, W], F32, isOutput=False)
    out = nc.declare_dram_parameter("out", [BP, 4], F32, isOutput=True)

    # (128, BP, 4, 512): partition p holds rows 4p..4p+3 of each image
    xv = x.ap().rearrange("b one (p a) w -> p (b one) a w", p=128)

    with TileContext(nc) as tc:
        with (
            tc.tile_pool(name="consts", bufs=1) as consts,
            tc.tile_pool(name="imgs", bufs=24) as imgs,
            tc.tile_pool(name="masks", bufs=6) as masks,
            tc.tile_pool(name="small", bufs=1) as small,
            tc.tile_pool(name="pcol", bufs=1, space="PSUM") as pcol_pool,
            tc.tile_pool(name="ptr", bufs=1, space="PSUM") as ptr_pool,
        ):
            psum_col = pcol_pool.tile([OHW, W], F32)
            oh = consts.tile([128, BP * OHW], BF16)
            ones_oh = consts.tile([128, BP * OHW], BF16)
            any_t = small.tile([128, 4 * BP], F32)
            rvals = small.tile([128, 2 * BP], F32)

            neg_half = consts.tile([128, 1], F32)
            hm_lo_i = consts.tile([128, GROUP * 4], I32)
            hm_lo = consts.tile([128, GROUP * 4], F32)
            hm_hi_i = consts.tile([128, GROUP * 4], I32)
            hm_hi = consts.tile([128, GROUP * 4], F32)
            wm_lo_i = consts.tile([BP, W], I32)
            wm_lo = consts.tile([BP, W], F32)
            wm_hi_i = consts.tile([BP, W], I32)
            wm_hi = consts.tile([BP, W], F32)
            ones128 = consts.tile([128, 128], F32)
            ident = consts.tile([128, 128], F32)
            dflt = consts.tile([BP, 4], F32)

            def emit_early_consts():
                nc.gpsimd.memset(neg_half[:], -0.5)
                # OH[p, i*OHW + i] = 1: routes image i to PSUM row i
                nc.gpsimd.memset(ones_oh[:], 1.0)
                nc.gpsimd.affine_select(
                    oh[:], ones_oh[:], [[-1, BP], [1, OHW]], OP.is_equal, 0.0,
                    base=0, channel_multiplier=0,
                )
                # row-index tables: y = 4p + r; lo = y - 512, hi = y + 1
                nc.gpsimd.iota(hm_lo_i[:], [[0, GROUP], [1, 4]], base=-512,
                               channel_multiplier=4)
                nc.gpsimd.tensor_copy(hm_lo[:], hm_lo_i[:])
                nc.gpsimd.iota(hm_hi_i[:], [[0, GROUP], [1, 4]], base=1,
                               channel_multiplier=4)
                nc.gpsimd.tensor_copy(hm_hi[:], hm_hi_i[:])

            def emit_tail_consts():
                # col-index tables: lo = j - 512, hi = j + 1
                nc.gpsimd.iota(wm_lo_i[:], [[1, W]], base=-512, channel_multiplier=0)
                nc.gpsimd.tensor_copy(wm_lo[:], wm_lo_i[:])
                nc.gpsimd.iota(wm_hi_i[:], [[1, W]], base=1, channel_multiplier=0)
                nc.gpsimd.tensor_copy(wm_hi[:], wm_hi_i[:])
                nc.gpsimd.memset(ones128[:], 1.0)
                nc.gpsimd.affine_select(
                    ident[:], ones128[:], [[-1, 128]], OP.is_equal, 0.0,
                    base=0, channel_multiplier=1,
                )
                nc.gpsimd.memset(dflt[:, 0:2], 0.25)
                nc.gpsimd.memset(dflt[:, 2:4], 0.75)

            # row-side groups; last group is a single image
            row_groups = [(0, 8), (8, 8), (16, 8), (24, 7), (31, 1)]

            def finish_group(start, n):
                cs = slice(4 * start, 4 * (start + n))
                rt_lo = small.tile([128, 4 * GROUP], F32, tag="rt_lo")
                nc.vector.scalar_tensor_tensor(
                    rt_lo[:, 0:4 * n], any_t[:, cs], ANY_THR, hm_lo[:, 0:4 * n],
                    OP.is_gt, OP.mult)
                nc.vector.tensor_reduce(
                    rvals[:, start:start + n],
                    rt_lo[:, 0:4 * n].rearrange("p (i r) -> p i r", r=4),
                    op=OP.min, axis=AX.X)
                rt_hi = small.tile([128, 4 * GROUP], F32, tag="rt_hi")
                nc.vector.scalar_tensor_tensor(
                    rt_hi[:, 0:4 * n], any_t[:, cs], ANY_THR, hm_hi[:, 0:4 * n],
                    OP.is_gt, OP.mult)
                nc.vector.tensor_reduce(
                    rvals[:, BP + start:BP + start + n],
                    rt_hi[:, 0:4 * n].rearrange("p (i r) -> p i r", r=4),
                    op=OP.max, axis=AX.X)

            # per-image row-block engine split: 'd' = DVE is_gt {0,1} count,
            # 'a' = ACT Relu(x-0.5) sum. For the last two images DVE takes the
            # late row blocks (it runs the rest of the tail anyway).
            def row_engines(i):
                if i >= BP - 2:
                    return "aadd"
                return "ddaa"

            def emit_dma(i):
                img = imgs.tile([128, IMG_FREE], BF16, tag="img")
                nc.gpsimd.dma_start(
                    out=img[:].rearrange("p (a w) -> p a w", a=4),
                    in_=xv[:, i:i + 1],
                )
                return img

            def emit_compute(i, img):
                m01 = masks.tile([128, IMG_FREE], BF16, tag="m01")
                for r, eng in enumerate(row_engines(i)):
                    sl = slice(r * W, (r + 1) * W)
                    acc = any_t[:, 4 * i + r:4 * i + r + 1]
                    if eng == "d":
                        nc.vector.tensor_scalar(
                            m01[:, sl], img[:, sl], 0.5, None,
                            OP.is_gt, OP.add, accum_out=acc)
                    else:
                        nc.scalar.activation(
                            m01[:, sl], img[:, sl], AF.Relu,
                            bias=neg_half[:], accum_out=acc)
                for r in range(4):
                    sl = slice(r * W, (r + 1) * W)
                    nc.tensor.matmul(
                        psum_col[:, :], oh[:, i * OHW:i * OHW + OHW], m01[:, sl],
                        start=(i == 0 and r == 0), stop=(i == BP - 1 and r == 3),
                    )
                for (gs, gn) in row_groups:
                    if gs + gn - 1 == i:
                        finish_group(gs, gn)

            img01 = [emit_dma(0), emit_dma(1)]
            emit_early_consts()
            emit_compute(0, img01[0])
            emit_compute(1, img01[1])
            for i in range(2, BP):
                if i == BP - 2:
                    emit_tail_consts()
                img = emit_dma(i)
                emit_compute(i, img)

            # ---- tail ----
            braw = small.tile([BP, 4], F32)

            # col side: (colsum > thr) * iota, masked min/max (rows 0..31)
            scr_lo = small.tile([BP, W], F32)
            scr_hi = small.tile([BP, W], F32)
            nc.vector.scalar_tensor_tensor(
                scr_lo[:], psum_col[0:BP, :], ANY_THR, wm_lo[:], OP.is_gt, OP.mult)
            nc.vector.tensor_reduce(braw[:, 0:1], scr_lo[:], op=OP.min, axis=AX.X)
            nc.vector.scalar_tensor_tensor(
                scr_hi[:], psum_col[0:BP, :], ANY_THR, wm_hi[:], OP.is_gt, OP.mult)
            nc.vector.tensor_reduce(braw[:, 2:3], scr_hi[:], op=OP.max, axis=AX.X)

            # row side: transpose rvals, reduce per image
            rT = ptr_pool.tile([2 * BP, 128], F32)
            nc.tensor.transpose(rT[:], rvals[:], ident[:])
            nc.vector.tensor_reduce(braw[:, 1:2], rT[0:BP, :], op=OP.min, axis=AX.X)
            nc.vector.tensor_reduce(braw[:, 3:4], rT[BP:2 * BP, :], op=OP.max, axis=AX.X)

            # empty mask: y_max raw is 0 iff no foreground
            emp = small.tile([BP, 1], F32)
            nc.vector.tensor_scalar(emp[:], braw[:, 3:4], 0.5, None, OP.is_lt)

            # normalize: lo = (v + 512)/512, hi = (v - 1)/512
            boxes = small.tile([BP, 4], F32)
            nc.vector.tensor_scalar(
                boxes[:, 0:2], braw[:, 0:2], 512.0, 1.0 / 512, OP.add, OP.mult)
            nc.vector.tensor_scalar(
                boxes[:, 2:4], braw[:, 2:4], 1.0, 1.0 / 512, OP.subtract, OP.mult)

            # expand-small fused: lo' = min(lo, max(0, c - s/2)),
            #                     hi' = max(hi, min(1, c + s/2))
            csum = small.tile([BP, 2], F32)
            lo2 = small.tile([BP, 2], F32)
            hi2 = small.tile([BP, 2], F32)
            final = small.tile([BP, 4], F32)
            nc.vector.tensor_add(csum[:], boxes[:, 0:2], boxes[:, 2:4])
            nc.vector.tensor_scalar(
                lo2[:], csum[:], 0.5, MIN_BOX * 0.5, OP.mult, OP.subtract)
            nc.vector.tensor_scalar(lo2[:], lo2[:], 0.0, None, OP.max)
            nc.vector.tensor_tensor(final[:, 0:2], boxes[:, 0:2], lo2[:], op=OP.min)
            nc.vector.tensor_scalar(
                hi2[:], csum[:], 0.5, MIN_BOX * 0.5, OP.mult, OP.add)
            nc.vector.tensor_scalar(hi2[:], hi2[:], 1.0, None, OP.min)
            nc.vector.tensor_tensor(final[:, 2:4], boxes[:, 2:4], hi2[:], op=OP.max)

            # default box where empty: final += (default - final) * emp
            dmb = small.tile([BP, 4], F32)
            nc.vector.tensor_sub(dmb[:], dflt[:], final[:])
            outb = small.tile([BP, 4], F32)
            nc.vector.scalar_tensor_tensor(
                outb[:], dmb[:], emp[:], final[:], OP.mult, OP.add)

            nc.sync.dma_start(out=out.ap(), in_=outb[:])

    return nc


_NC = None


def _get_nc():
    global _NC
    if _NC is None:
        nc = build_nc()
        nc.compile()
        _NC = nc
    return _NC


def kernel(mask_fg: np.ndarray) -> np.ndarray:
    mask_fg = np.ascontiguousarray(np.asarray(mask_fg, dtype=np.float32))
    assert mask_fg.shape == (B, 1, H, W), mask_fg.shape
    nc = _get_nc()
    shards = mask_fg.reshape(N_CORES, BP, 1, H, W)
    in_maps = [{"mask_fg": np.ascontiguousarray(shards[i])} for i in range(N_CORES)]
    res = run_bass_kernel_spmd(nc, in_maps, core_ids=list(range(N_CORES)))
    return np.concatenate(
        [res.results[i]["out"] for i in range(N_CORES)], axis=0
    ).astype(np.float32)


# revision 21
# speedup vs baseline: 1.2131x; 1.1988x over previous
"""Trainium2 Bass kernel for nn_BBoxGenerator (segment_reduce).

mask_fg (256, 1, 512, 512) f32 -> boxes (256, 4) f32 [x0, y0, x1, y1].

Pure data parallel: each of the 8 cores handles 32 images independently.

v8: HWDGE (nc.sync) f32 stream + reworked compute/finishing.
  Why HWDGE: SWDGE descriptor generation runs on GpSimd's Q7 and its
  descriptor rings live in SBUF; DVE tensor_scalar ops (our per-image
  threshold) enter 2-port perf mode which blocks GpSimd, and the ring
  fetches contend on the AXI ports serving SDMA engines 7/15 - traced
  as a sustained +16% per-descriptor slowdown on DMA_15 that paced the
  whole stream (v3-v7, 120-126us). HWDGE has no SBUF rings and never
  contends with DVE; the cost is losing the f32->bf16 cast-during-DMA,
  so images land as f32 (16 x 1 MiB SBUF buffers) and the threshold
  ops read f32.
  - Unified "positive iff any" masks: DVE rows use (x > 0.5) {0,1} with
    fused accum row sums; ACT rows use Relu(x-0.5) >= 0 with fused accum
    row sums. Row sums and PSUM column sums are all "> 0.002 iff any
    foreground": one threshold everywhere, PSUM rows 0..31 in image
    order, ONE output DMA (no un-permute).
  - Row side: per-group masked-iota min/max into rvals during the
    stream; one PE transpose + two small reduces at the tail.
  - Col side: fused (colsum > thr) * iota via scalar_tensor_tensor +
    tensor_reduce min/max. (tensor_tensor_reduce would fuse further but
    crashes the HW: NRT_EXEC_UNIT_UNRECOVERABLE, micro_test.py stage 5.)
  - Box expand fused without predicates:
    lo' = min(lo, max(0, c - s/2)), hi' = max(hi, min(1, c + s/2)).
  - Q7 (gpsimd) paces SWDGE descriptor emission ring-full, so const
    emission is split: tiny block (one-hot + row tables) right after
    dma1 while the ring is shallow; tail-only consts before dma30 where
    the deep ring absorbs the pause.
"""

import numpy as np

from concourse import bacc, mybir
from concourse.tile import TileContext
from concourse.bass_utils import run_bass_kernel_spmd

F32 = mybir.dt.float32
BF16 = mybir.dt.bfloat16
I32 = mybir.dt.int32
OP = mybir.AluOpType
AX = mybir.AxisListType
AF = mybir.ActivationFunctionType

N_CORES = 8
B = 256
BP = B // N_CORES  # 32 images per core
H = W = 512
IMG_FREE = 4 * W  # 2048 free elems per image (4 rows per partition)
GROUP = 8
OHW = 64  # one-hot block width per image (v1 width; rows 32..63 unused)

MIN_BOX = 0.05
ANY_THR = 0.002  # any_t/PSUM sums exceed this iff any foreground


def build_nc():
    nc = bacc.Bacc("TRN2", target_bir_lowering=False, debug=False, num_devices=N_CORES)
    x = nc.declare_dram_parameter("mask_fg", [BP, 1, H, W], F32, isOutput=False)
    out = nc.declare_dram_parameter("out", [BP, 4], F32, isOutput=True)

    # (128, BP, 4, 512): partition p holds rows 4p..4p+3 of each image
    xv = x.ap().rearrange("b one (p a) w -> p (b one) a w", p=128)

    with TileContext(nc) as tc:
        with (
            tc.tile_pool(name="consts", bufs=1) as consts,
            tc.tile_pool(name="imgs", bufs=16) as imgs,
            tc.tile_pool(name="masks", bufs=6) as masks,
            tc.tile_pool(name="small", bufs=1) as small,
            tc.tile_pool(name="pcol", bufs=1, space="PSUM") as pcol_pool,
            tc.tile_pool(name="ptr", bufs=1, space="PSUM") as ptr_pool,
        ):
            psum_col = pcol_pool.tile([OHW, W], F32)
            oh = consts.tile([128, BP * OHW], BF16)
            ones_oh = consts.tile([128, BP * OHW], BF16)
            any_t = small.tile([128, 4 * BP], F32)
            rvals = small.tile([128, 2 * BP], F32)

            neg_half = consts.tile([128, 1], F32)
            hm_lo_i = consts.tile([128, GROUP * 4], I32)
            hm_lo = consts.tile([128, GROUP * 4], F32)
            hm_hi_i = consts.tile([128, GROUP * 4], I32)
            hm_hi = consts.tile([128, GROUP * 4], F32)
            wm_lo_i = consts.tile([BP, W], I32)
            wm_lo = consts.tile([BP, W], F32)
            wm_hi_i = consts.tile([BP, W], I32)
            wm_hi = consts.tile([BP, W], F32)
            ones128 = consts.tile([128, 128], F32)
            ident = consts.tile([128, 128], F32)
            dflt = consts.tile([BP, 4], F32)

            def emit_early_consts():
                nc.gpsimd.memset(neg_half[:], -0.5)
                # OH[p, i*OHW + i] = 1: routes image i to PSUM row i
                nc.gpsimd.memset(ones_oh[:], 1.0)
                nc.gpsimd.affine_select(
                    oh[:], ones_oh[:], [[-1, BP], [1, OHW]], OP.is_equal, 0.0,
                    base=0, channel_multiplier=0,
                )
                # row-index tables: y = 4p + r; lo = y - 512, hi = y + 1
                nc.gpsimd.iota(hm_lo_i[:], [[0, GROUP], [1, 4]], base=-512,
                               channel_multiplier=4)
                nc.gpsimd.tensor_copy(hm_lo[:], hm_lo_i[:])
                nc.gpsimd.iota(hm_hi_i[:], [[0, GROUP], [1, 4]], base=1,
                               channel_multiplier=4)
                nc.gpsimd.tensor_copy(hm_hi[:], hm_hi_i[:])

            def emit_tail_consts():
                # col-index tables: lo = j - 512, hi = j + 1
                nc.gpsimd.iota(wm_lo_i[:], [[1, W]], base=-512, channel_multiplier=0)
                nc.gpsimd.tensor_copy(wm_lo[:], wm_lo_i[:])
                nc.gpsimd.iota(wm_hi_i[:], [[1, W]], base=1, channel_multiplier=0)
                nc.gpsimd.tensor_copy(wm_hi[:], wm_hi_i[:])
                nc.gpsimd.memset(ones128[:], 1.0)
                nc.gpsimd.affine_select(
                    ident[:], ones128[:], [[-1, 128]], OP.is_equal, 0.0,
                    base=0, channel_multiplier=1,
                )
                nc.gpsimd.memset(dflt[:, 0:2], 0.25)
                nc.gpsimd.memset(dflt[:, 2:4], 0.75)

            # row-side groups; last group is a single image
            row_groups = [(0, 8), (8, 8), (16, 8), (24, 7), (31, 1)]

            def finish_group(start, n):
                cs = slice(4 * start, 4 * (start + n))
                rt_lo = small.tile([128, 4 * GROUP], F32, tag="rt_lo")
                nc.vector.scalar_tensor_tensor(
                    rt_lo[:, 0:4 * n], any_t[:, cs], ANY_THR, hm_lo[:, 0:4 * n],
                    OP.is_gt, OP.mult)
                nc.vector.tensor_reduce(
                    rvals[:, start:start + n],
                    rt_lo[:, 0:4 * n].rearrange("p (i r) -> p i r", r=4),
                    op=OP.min, axis=AX.X)
                rt_hi = small.tile([128, 4 * GROUP], F32, tag="rt_hi")
                nc.vector.scalar_tensor_tensor(
                    rt_hi[:, 0:4 * n], any_t[:, cs], ANY_THR, hm_hi[:, 0:4 * n],
                    OP.is_gt, OP.mult)
                nc.vector.tensor_reduce(
                    rvals[:, BP + start:BP + start + n],
                    rt_hi[:, 0:4 * n].rearrange("p (i r) -> p i r", r=4),
                    op=OP.max, axis=AX.X)

            # per-image row-block engine split: 'd' = DVE is_gt {0,1} count,
            # 'a' = ACT Relu(x-0.5) sum. For the last two images DVE takes the
            # late row blocks (it runs the rest of the tail anyway).
            def row_engines(i):
                if i >= BP - 2:
                    return "aadd"
                return "ddaa"

            def emit_dma(i):
                img = imgs.tile([128, IMG_FREE], F32, tag="img")
                nc.sync.dma_start(
                    out=img[:].rearrange("p (a w) -> p a w", a=4),
                    in_=xv[:, i:i + 1],
                )
                return img

            def emit_compute(i, img):
                m01 = masks.tile([128, IMG_FREE], BF16, tag="m01")
                for r, eng in enumerate(row_engines(i)):
                    sl = slice(r * W, (r + 1) * W)
                    acc = any_t[:, 4 * i + r:4 * i + r + 1]
                    if eng == "d":
                        nc.vector.tensor_scalar(
                            m01[:, sl], img[:, sl], 0.5, None,
                            OP.is_gt, OP.add, accum_out=acc)
                    else:
                        nc.scalar.activation(
                            m01[:, sl], img[:, sl], AF.Relu,
                            bias=neg_half[:], accum_out=acc)
                for r in range(4):
                    sl = slice(r * W, (r + 1) * W)
                    nc.tensor.matmul(
                        psum_col[:, :], oh[:, i * OHW:i * OHW + OHW], m01[:, sl],
                        start=(i == 0 and r == 0), stop=(i == BP - 1 and r == 3),
                    )
                for (gs, gn) in row_groups:
                    if gs + gn - 1 == i:
                        finish_group(gs, gn)

            emit_early_consts()
            emit_tail_consts()
            for i in range(BP):
                img = emit_dma(i)
                emit_compute(i, img)

            # ---- tail ----
            braw = small.tile([BP, 4], F32)

            # col side: (colsum > thr) * iota, masked min/max (rows 0..31)
            scr_lo = small.tile([BP, W], F32)
            scr_hi = small.tile([BP, W], F32)
            nc.vector.scalar_tensor_tensor(
                scr_lo[:], psum_col[0:BP, :], ANY_THR, wm_lo[:], OP.is_gt, OP.mult)
            nc.vector.tensor_reduce(braw[:, 0:1], scr_lo[:], op=OP.min, axis=AX.X)
            nc.vector.scalar_tensor_tensor(
                scr_hi[:], psum_col[0:BP, :], ANY_THR, wm_hi[:], OP.is_gt, OP.mult)
            nc.vector.tensor_reduce(braw[:, 2:3], scr_hi[:], op=OP.max, axis=AX.X)

            # row side: transpose rvals, reduce per image
            rT = ptr_pool.tile([2 * BP, 128], F32)
            nc.tensor.transpose(rT[:], rvals[:], ident[:])
            nc.vector.tensor_reduce(braw[:, 1:2], rT[0:BP, :], op=OP.min, axis=AX.X)
            nc.vector.tensor_reduce(braw[:, 3:4], rT[BP:2 * BP, :], op=OP.max, axis=AX.X)

            # empty mask: y_max raw is 0 iff no foreground
            emp = small.tile([BP, 1], F32)
            nc.vector.tensor_scalar(emp[:], braw[:, 3:4], 0.5, None, OP.is_lt)

            # normalize: lo = (v + 512)/512, hi = (v - 1)/512
            boxes = small.tile([BP, 4], F32)
            nc.vector.tensor_scalar(
                boxes[:, 0:2], braw[:, 0:2], 512.0, 1.0 / 512, OP.add, OP.mult)
            nc.vector.tensor_scalar(
                boxes[:, 2:4], braw[:, 2:4], 1.0, 1.0 / 512, OP.subtract, OP.mult)

            # expand-small fused: lo' = min(lo, max(0, c - s/2)),
            #                     hi' = max(hi, min(1, c + s/2))
            csum = small.tile([BP, 2], F32)
            lo2 = small.tile([BP, 2], F32)
            hi2 = small.tile([BP, 2], F32)
            final = small.tile([BP, 4], F32)
            nc.vector.tensor_add(csum[:], boxes[:, 0:2], boxes[:, 2:4])
            nc.vector.tensor_scalar(
                lo2[:], csum[:], 0.5, MIN_BOX * 0.5, OP.mult, OP.subtract)
            nc.vector.tensor_scalar(lo2[:], lo2[:], 0.0, None, OP.max)
            nc.vector.tensor_tensor(final[:, 0:2], boxes[:, 0:2], lo2[:], op=OP.min)
            nc.vector.tensor_scalar(
                hi2[:], csum[:], 0.5, MIN_BOX * 0.5, OP.mult, OP.add)
            nc.vector.tensor_scalar(hi2[:], hi2[:], 1.0, None, OP.min)
            nc.vector.tensor_tensor(final[:, 2:4], boxes[:, 2:4], hi2[:], op=OP.max)

            # default box where empty: final += (default - final) * emp
            dmb = small.tile([BP, 4], F32)
            nc.vector.tensor_sub(dmb[:], dflt[:], final[:])
            outb = small.tile([BP, 4], F32)
            nc.vector.scalar_tensor_tensor(
                outb[:], dmb[:], emp[:], final[:], OP.mult, OP.add)

            nc.sync.dma_start(out=out.ap(), in_=outb[:])

    return nc


_NC = None


def _get_nc():
    global _NC
    if _NC is None:
        nc = build_nc()
        nc.compile()
        _NC = nc
    return _NC


def kernel(mask_fg: np.ndarray) -> np.ndarray:
    mask_fg = np.ascontiguousarray(np.asarray(mask_fg, dtype=np.float32))
    assert mask_fg.shape == (B, 1, H, W), mask_fg.shape
    nc = _get_nc()
    shards = mask_fg.reshape(N_CORES, BP, 1, H, W)
    in_maps = [{"mask_fg": np.ascontiguousarray(shards[i])} for i in range(N_CORES)]
    res = run_bass_kernel_spmd(nc, in_maps, core_ids=list(range(N_CORES)))
    return np.concatenate(
        [res.results[i]["out"] for i in range(N_CORES)], axis=0
    ).astype(np.float32)
